# revision 1
# baseline (speedup 1.0000x reference)
"""GCNConv on 8 Trainium2 NeuronCores (Bass/Tile, SPMD).

out = D^-1/2 (A+I) D^-1/2 (X @ W.T),   deg = in-degree(col) + 1

Math refactoring (exact in real arithmetic):
    agg[r]  = sum_{e: dst=r} d[col_e] * X[col_e]      (self loop = edge (r,r))
    out[r]  = d[r] * (agg[r] @ W.T)                   (d = deg^-1/2)

Distribution: destinations (rows) are sharded across the 8 cores (12500
each); each core processes the edges whose destination lands in its shard.
X and W are replicated so any core can read any source row.

Device algorithm per core (one SPMD program; per-core index tables are
padded into a common, max-over-cores structure so SPMD is preserved):

  * Edge slots: edges (+ self loops) are grouped into segments by (range of
    RNG_DTS dest-tiles, source-chunk c of 25000 rows), sorted by destination
    inside each segment and packed densely (slot i of a gather lives at SBUF
    partition i%128, free block i//128).  Trailing pad entries use index -1
    and a per-core valid-count register, so padding costs zero descriptors.
  * Gather: one `dma_gather` (int16 indices relative to the chunk base) per
    segment pulls the 512-byte X rows of its slots.  This dominates the
    runtime and is descriptor-latency-bound (~9 ns/row aggregate), hence the
    dense packing: descriptors == real edges only.
  * Scale: one DVE multiply per segment applies d[col] per slot
    (per-partition scalar broadcast over the 128 features); d is computed on
    device from the integer degree counts (ACT sqrt + DVE reciprocal).
  * Segmented sum via PE: per 128-slot tile, a host-built 0/1 selection
    matrix S (slots x dests, bf16 in DRAM, cast to f32 during the DMA load)
    routes slots to destinations: psum[feat, dest] += g_tile.T @ S_tile,
    accumulating into a range-wide one-bank PSUM tile [128, RNG_DTS*128].
  * Finalize per dest-tile: copy PSUM->SBUF, matmul with W.T (contraction
    over features), scale by d[dest] (per-partition scalar), DMA out.

The host does index marshaling only (bucketing, sorting, degree counts, 0/1
selection structure); all floating-point math on X/W runs on device.
"""

import math

import numpy as np
import ml_dtypes

import concourse.bacc as bacc
import concourse.mybir as mybir
import concourse.tile as tile
from concourse.bass_utils import run_bass_kernel_spmd
from concourse import library_config

NCORES = 8
P = 128
CH_SPAN = 25000          # source rows per gather chunk (int16-indexable)
RNG_DTS = 4              # dest-tiles per range (psum tile = 1 bank = 512 f32)
DEG_PAD = 1.0e30         # pad degree -> d ~ 0

F32 = mybir.dt.float32
BF16 = mybir.dt.bfloat16
I16 = mybir.dt.int16


class Plan:
    pass


# ----------------------------------------------------------------------------
# Host-side index marshaling
# ----------------------------------------------------------------------------

def _preprocess(edge_index: np.ndarray, n_nodes: int):
    ns = n_nodes // NCORES
    rt = math.ceil(ns / P)
    nch = math.ceil(n_nodes / CH_SPAN)
    nrng = math.ceil(rt / RNG_DTS)

    row = np.asarray(edge_index[0]).astype(np.int64)
    col = np.asarray(edge_index[1]).astype(np.int64)
    deg = (np.bincount(col, minlength=n_nodes) + 1).astype(np.float32)

    core = row // ns
    cores = []
    for m in range(NCORES):
        sel = core == m
        r_l = row[sel] - m * ns
        c_g = col[sel]
        r_l = np.concatenate([r_l, np.arange(ns, dtype=np.int64)])
        c_g = np.concatenate([c_g, np.arange(ns, dtype=np.int64) + m * ns])
        rg = r_l // (RNG_DTS * P)
        ch = np.minimum(c_g // CH_SPAN, nch - 1)
        order = np.lexsort((c_g, r_l, ch, rg))
        r_l, c_g = r_l[order], c_g[order]
        code = rg[order] * nch + ch[order]
        bounds = np.searchsorted(code, np.arange(nrng * nch + 1))
        cores.append(dict(r_l=r_l, c_g=c_g, bounds=bounds))

    # segment tile counts: max over cores (packed, no per-dt padding)
    plan = Plan()
    plan.ns, plan.rt, plan.nch, plan.nrng = ns, rt, nch, nrng
    plan.segs = []
    jtot = 0
    for rg in range(nrng):
        for c in range(nch):
            g = rg * nch + c
            ntiles = 0
            for m in range(NCORES):
                b = cores[m]["bounds"]
                ntiles = max(ntiles, (int(b[g + 1] - b[g]) + P - 1) // P)
            if ntiles == 0:
                continue
            plan.segs.append(dict(base=c * CH_SPAN, t16_0=jtot * 8,
                                  n=ntiles * P, j0=jtot, ntiles=ntiles,
                                  rng=rg, c=c, g=g))
            jtot += ntiles
    plan.jtot = jtot
    plan.tot16 = jtot * 8

    nslots = jtot * P
    gidx = np.zeros((NCORES, P, plan.tot16), np.int16)
    deg_col = np.full((NCORES, nslots), DEG_PAD, np.float32)
    dest_arr = np.full((NCORES, nslots), -1, np.int64)  # rel to range base
    cnts = np.zeros((NCORES, max(1, len(plan.segs))), np.int32)
    for m in range(NCORES):
        r_l, c_g, b = cores[m]["r_l"], cores[m]["c_g"], cores[m]["bounds"]
        idx16 = np.full(nslots, -1, np.int16)
        for si, seg in enumerate(plan.segs):
            g = seg["g"]
            lo, hi = int(b[g]), int(b[g + 1])
            n = hi - lo
            if n == 0:
                # still need >= 1 valid index (dummy row 0, zero S row)
                idx16[seg["j0"] * P] = 0
                cnts[m, si] = 1
                continue
            cnts[m, si] = n
            s0 = seg["j0"] * P
            cg = c_g[lo:hi]
            idx16[s0:s0 + n] = (cg - seg["base"]).astype(np.int16)
            deg_col[m, s0:s0 + n] = deg[cg]
            dest_arr[m, s0:s0 + n] = (r_l[lo:hi]
                                      - seg["rng"] * RNG_DTS * P)
        w = idx16.reshape(plan.tot16, 16).T
        gidx[m] = np.tile(w, (8, 1))

    # common per-tile S frames (dmin/nd = union over cores, within the
    # RNG_DTS*128-wide range -> nd <= 512 always)
    da = dest_arr.reshape(NCORES, jtot, P)
    da_min = np.where(da < 0, 10 ** 9, da).min(axis=(0, 2))
    da_max = da.max(axis=(0, 2))
    plan.dmin = da_min.astype(np.int64)
    plan.nd = (da_max - da_min + 1).astype(np.int64)
    assert (plan.nd >= 1).all() and (plan.nd <= RNG_DTS * P).all()
    plan.s0 = np.zeros(jtot + 1, np.int64)
    np.cumsum(plan.nd, out=plan.s0[1:])
    plan.stot = int(plan.s0[-1])

    # S matrices
    s_pack = np.zeros((NCORES, P, plan.stot), ml_dtypes.bfloat16)
    ar = np.arange(P)
    for m in range(NCORES):
        for t in range(jtot):
            dl = da[m, t]
            v = dl >= 0
            if not v.any():
                continue
            blk = np.zeros((P, int(plan.nd[t])), np.float32)
            blk[ar[v], dl[v] - plan.dmin[t]] = 1.0
            s_pack[m, :, plan.s0[t]:plan.s0[t + 1]] = blk

    deg_nat = np.full((NCORES, P, rt), 1.0, np.float32)
    for m in range(NCORES):
        d = np.full(rt * P, 1.0, np.float32)
        d[:ns] = deg[m * ns:(m + 1) * ns]
        deg_nat[m] = d.reshape(rt, P).T

    deg_col = deg_col.reshape(NCORES, jtot, P).transpose(0, 2, 1)

    # per-range tile spans (S streamed per range)
    plan.rng_tiles = []
    seg_by_rng = {}
    for si, seg in enumerate(plan.segs):
        seg_by_rng.setdefault(seg["rng"], []).append(si)
    t = 0
    for rg in range(nrng):
        t0 = t
        for si in seg_by_rng.get(rg, []):
            t += plan.segs[si]["ntiles"]
        plan.rng_tiles.append((t0, t))
    plan.seg_by_rng = seg_by_rng
    plan.swmax = max((int(plan.s0[t1] - plan.s0[t0])
                      for t0, t1 in plan.rng_tiles if t1 > t0), default=1)
    plan.nmax = max(s["n"] for s in plan.segs)

    data = dict(gidx=gidx, deg_col=np.ascontiguousarray(deg_col),
                s_pack=s_pack, deg_nat=deg_nat, cnts=cnts)
    return plan, data


# ----------------------------------------------------------------------------
# Device program (identical for all cores)
# ----------------------------------------------------------------------------

def _build_nc(n_nodes: int, plan: Plan):
    ns, rt, nch, nrng = plan.ns, plan.rt, plan.nch, plan.nrng
    nc = bacc.Bacc("TRN2", target_bir_lowering=False, debug=False,
                   num_devices=NCORES)

    x_d = nc.dram_tensor("x", [n_nodes, P], F32, kind="ExternalInput").ap()
    wt_d = nc.dram_tensor("wt", [P, P], F32, kind="ExternalInput").ap()
    gix_d = nc.dram_tensor("gidx", [P, plan.tot16], I16,
                           kind="ExternalInput").ap()
    dcol_d = nc.dram_tensor("deg_col", [P, plan.jtot], F32,
                            kind="ExternalInput").ap()
    dnat_d = nc.dram_tensor("deg_nat", [P, rt], F32,
                            kind="ExternalInput").ap()
    s_d = nc.dram_tensor("s_pack", [P, plan.stot], BF16,
                         kind="ExternalInput").ap()
    cnt_d = nc.dram_tensor("cnts", [1, max(1, len(plan.segs))],
                           mybir.dt.int32, kind="ExternalInput").ap()
    out_d = nc.dram_tensor("out", [rt * P, P], F32, kind="ExternalOutput").ap()

    pw = RNG_DTS * P
    with tile.TileContext(nc) as tc:
        nc.gpsimd.load_library(library_config.mlp)
        with (
            tc.tile_pool(name="const", bufs=1) as cpool,
            tc.tile_pool(name="gbuf", bufs=3) as gpool,
            tc.tile_pool(name="sbuf_s", bufs=2) as spool,
            tc.tile_pool(name="fin", bufs=4) as fpool,
            tc.tile_pool(name="pacc", bufs=4, space="PSUM") as papool,
            tc.tile_pool(name="pout", bufs=2, space="PSUM") as popool,
        ):
            wt_sb = cpool.tile([P, P], F32)
            nc.sync.dma_start(out=wt_sb[:], in_=wt_d[:, :])
            gidx_sb = cpool.tile([P, plan.tot16], I16)
            nc.sync.dma_start(out=gidx_sb[:], in_=gix_d[:, :])

            dcol_sb = cpool.tile([P, plan.jtot], F32)
            nc.sync.dma_start(out=dcol_sb[:], in_=dcol_d[:, :])
            nc.scalar.activation(dcol_sb[:], dcol_sb[:],
                                 mybir.ActivationFunctionType.Sqrt)
            d_col = cpool.tile([P, plan.jtot], F32)
            nc.vector.reciprocal(d_col[:], dcol_sb[:])

            dnat_sb = cpool.tile([P, rt], F32)
            nc.sync.dma_start(out=dnat_sb[:], in_=dnat_d[:, :])
            nc.scalar.activation(dnat_sb[:], dnat_sb[:],
                                 mybir.ActivationFunctionType.Sqrt)
            d_nat = cpool.tile([P, rt], F32)
            nc.vector.reciprocal(d_nat[:], dnat_sb[:])

            zcol = cpool.tile([1, P], BF16)
            nc.vector.memset(zcol[:], 0.0)
            zrow = cpool.tile([1, pw], BF16)
            nc.vector.memset(zrow[:], 0.0)

            cnt_sb = cpool.tile([1, max(1, len(plan.segs))], mybir.dt.int32)
            nc.sync.dma_start(out=cnt_sb[:], in_=cnt_d[:, :])
            cnt_regs = [nc.gpsimd.alloc_register(f"cntr{i}") for i in range(4)]

            for rg in range(nrng):
                t0, t1 = plan.rng_tiles[rg]
                if t1 == t0:
                    continue
                sw0, sw1 = int(plan.s0[t0]), int(plan.s0[t1])
                s_sb = spool.tile([P, plan.swmax], F32, tag="s_sb")
                # bf16 -> f32 cast during DMA (SWDGE)
                nc.gpsimd.dma_start(out=s_sb[:, :sw1 - sw0],
                                    in_=s_d[:, sw0:sw1])

                pt = papool.tile([P, pw], F32, tag="pacc")
                nc.tensor.matmul(pt[:], lhsT=zcol[:], rhs=zrow[:],
                                 start=True, stop=False,
                                 skip_group_check=True)

                segs_rng = plan.seg_by_rng.get(rg, [])
                for k, si in enumerate(segs_rng):
                    seg = plan.segs[si]
                    jseg, nseg = seg["ntiles"], seg["n"]
                    g = gpool.tile([P, plan.nmax], F32, tag="g")
                    g3 = g[:, :nseg].rearrange("p (j f) -> p j f", f=P)
                    # pad slots are skipped by the gather (idx -1); zero them
                    # so the scale/matmuls see no stale garbage
                    nc.vector.memset(g[:, :nseg], 0.0)
                    span = min(CH_SPAN, n_nodes - seg["base"])
                    creg = cnt_regs[si % len(cnt_regs)]
                    nc.gpsimd.reg_load(creg, cnt_sb[0:1, si:si + 1])
                    nc.gpsimd.dma_gather(
                        g3, x_d[seg["base"]:seg["base"] + span, :],
                        gidx_sb[:, seg["t16_0"]:seg["t16_0"] + jseg * 8],
                        nseg, creg, P, single_packet=False,
                    )
                    dsl = d_col[:, seg["j0"]:seg["j0"] + jseg]
                    nc.vector.tensor_mul(
                        g3, g3, dsl[:, :, None].to_broadcast([P, jseg, P]))
                    for jj in range(jseg):
                        t = seg["j0"] + jj
                        dmin, nd = int(plan.dmin[t]), int(plan.nd[t])
                        sa = int(plan.s0[t]) - sw0
                        is_last = (k == len(segs_rng) - 1 and jj == jseg - 1)
                        nc.tensor.matmul(
                            pt[:, dmin:dmin + nd],
                            lhsT=g[:, jj * P:(jj + 1) * P],
                            rhs=s_sb[:, sa:sa + nd],
                            start=False, stop=is_last,
                            skip_group_check=True,
                        )

                for dl in range(min(RNG_DTS, rt - rg * RNG_DTS)):
                    dt = rg * RNG_DTS + dl
                    aggt = fpool.tile([P, P], F32, tag="aggt")
                    nc.vector.tensor_copy(aggt[:], pt[:, dl * P:(dl + 1) * P])
                    op = popool.tile([P, P], F32, tag="op")
                    nc.tensor.matmul(op[:], lhsT=aggt[:], rhs=wt_sb[:],
                                     start=True, stop=True)
                    ob = fpool.tile([P, P], F32, tag="ob")
                    nc.vector.tensor_scalar_mul(ob[:], op[:],
                                                d_nat[:, dt:dt + 1])
                    nc.sync.dma_start(out=out_d[dt * P:(dt + 1) * P, :],
                                      in_=ob[:])
    nc.compile()
    return nc


# ----------------------------------------------------------------------------
# Entry point
# ----------------------------------------------------------------------------

_CACHE: dict = {}


def _prepare(X, W, edge_index):
    X = np.ascontiguousarray(np.asarray(X, dtype=np.float32))
    W = np.asarray(W, dtype=np.float32)
    edge_index = np.asarray(edge_index)
    n = X.shape[0]
    plan, data = _preprocess(edge_index, n)
    key = (n, plan.jtot, plan.stot, tuple(s["n"] for s in plan.segs))
    if key not in _CACHE:
        _CACHE.clear()
        _CACHE[key] = _build_nc(n, plan)
    nc = _CACHE[key]
    wt = np.ascontiguousarray(W.T)
    in_maps = [
        {
            "x": X,
            "wt": wt,
            "gidx": np.ascontiguousarray(data["gidx"][m]),
            "deg_col": np.ascontiguousarray(data["deg_col"][m]),
            "deg_nat": np.ascontiguousarray(data["deg_nat"][m]),
            "s_pack": np.ascontiguousarray(data["s_pack"][m]),
            "cnts": np.ascontiguousarray(data["cnts"][m][None, :]),
        }
        for m in range(NCORES)
    ]
    return nc, in_maps, plan


def kernel(X, W, edge_index):
    nc, in_maps, plan = _prepare(X, W, edge_index)
    res = run_bass_kernel_spmd(nc, in_maps, core_ids=list(range(NCORES)))
    ns = plan.ns
    return np.concatenate([res.results[m]["out"][:ns] for m in range(NCORES)],
                          axis=0)



# revision 2
# speedup vs baseline: 1.9998x; 1.9998x over previous
"""GCNConv on 8 Trainium2 NeuronCores (Bass/Tile, SPMD).

out = D^-1/2 (A+I) D^-1/2 (X @ W.T),   deg = in-degree(col) + 1

Math refactoring (exact in real arithmetic):
    agg[r]  = sum_{e: dst=r} d[col_e] * X[col_e]      (self loop = edge (r,r))
    out[r]  = d[r] * (agg[r] @ W.T)                   (d = deg^-1/2)

Distribution: destinations (rows) are sharded across the 8 cores (12500
each); each core processes the edges whose destination lands in its shard.
X and W are replicated so any core can read any source row.

Device algorithm per core (one SPMD program; per-core index tables are
padded into a common, max-over-cores structure so SPMD is preserved):

  * Edge slots: edges (+ self loops) are grouped into segments by (range of
    RNG_DTS dest-tiles, source-chunk c of 25000 rows), sorted by destination
    inside each segment and packed densely (slot i of a gather lives at SBUF
    partition i%128, free block i//128).  Trailing pad entries use index -1
    and a per-core valid-count register, so padding costs zero descriptors.
  * Gather: one `dma_gather` (int16 indices relative to the chunk base) per
    segment pulls the 512-byte X rows of its slots.  The per-descriptor cost
    is a per-SWDGE-queue drain wall (~9 ns/desc on one queue); the gathers
    round-robin over all 4 SWDGE queues, which overlaps their drains and
    brings the aggregate rate to ~2.3 ns/desc.
  * Scale: one DVE multiply per segment applies d[col] per slot
    (per-partition scalar broadcast over the 128 features), writing a bf16
    copy of the tile; d is computed on device from the integer degree counts
    (ACT sqrt + DVE reciprocal).  Pad slots are killed by zero S rows (and a
    one-time buffer memset guarantees no NaN garbage on first use).
  * Segmented sum via PE: per 128-slot tile, a host-built 0/1 selection
    matrix S (slots x dests, bf16) routes slots to destinations:
    psum[feat, dest] += g_tile.T @ S_tile (both operands bf16, 1 cycle/row),
    accumulating into a range-wide one-bank PSUM tile [128, RNG_DTS*128].
  * Finalize per dest-tile: copy PSUM->SBUF (bf16), matmul with W.T
    (bf16, contraction over features), scale by d[dest] (per-partition
    scalar, f32), DMA out.

The host does index marshaling only (bucketing, sorting, degree counts, 0/1
selection structure); all floating-point math on X/W runs on device.
"""

import math

import numpy as np
import ml_dtypes

import concourse.bacc as bacc
import concourse.mybir as mybir
import concourse.tile as tile
from concourse.bass_utils import run_bass_kernel_spmd
from concourse import library_config

NCORES = 8
P = 128
CH_SPAN = 25000          # source rows per gather chunk (int16-indexable)
RNG_DTS = 4              # dest-tiles per range (psum tile = 1 bank = 512 f32)
NQ = 4                   # SWDGE queues (gather drains overlap across queues)
DEG_PAD = 1.0e30         # pad degree -> d ~ 0

F32 = mybir.dt.float32
BF16 = mybir.dt.bfloat16
I16 = mybir.dt.int16


class Plan:
    pass


# ----------------------------------------------------------------------------
# Host-side index marshaling
# ----------------------------------------------------------------------------

def _preprocess(edge_index: np.ndarray, n_nodes: int):
    ns = n_nodes // NCORES
    rt = math.ceil(ns / P)
    nch = math.ceil(n_nodes / CH_SPAN)
    nrng = math.ceil(rt / RNG_DTS)

    row = np.asarray(edge_index[0]).astype(np.int64)
    col = np.asarray(edge_index[1]).astype(np.int64)
    deg = (np.bincount(col, minlength=n_nodes) + 1).astype(np.float32)

    core = row // ns
    cores = []
    for m in range(NCORES):
        sel = core == m
        r_l = row[sel] - m * ns
        c_g = col[sel]
        r_l = np.concatenate([r_l, np.arange(ns, dtype=np.int64)])
        c_g = np.concatenate([c_g, np.arange(ns, dtype=np.int64) + m * ns])
        rg = r_l // (RNG_DTS * P)
        ch = np.minimum(c_g // CH_SPAN, nch - 1)
        order = np.lexsort((c_g, r_l, ch, rg))
        r_l, c_g = r_l[order], c_g[order]
        code = rg[order] * nch + ch[order]
        bounds = np.searchsorted(code, np.arange(nrng * nch + 1))
        cores.append(dict(r_l=r_l, c_g=c_g, bounds=bounds))

    # segment tile counts: max over cores (packed, no per-dt padding)
    plan = Plan()
    plan.ns, plan.rt, plan.nch, plan.nrng = ns, rt, nch, nrng
    plan.segs = []
    jtot = 0
    for rg in range(nrng):
        for c in range(nch):
            g = rg * nch + c
            ntiles = 0
            for m in range(NCORES):
                b = cores[m]["bounds"]
                ntiles = max(ntiles, (int(b[g + 1] - b[g]) + P - 1) // P)
            if ntiles == 0:
                continue
            plan.segs.append(dict(base=c * CH_SPAN, t16_0=jtot * 8,
                                  n=ntiles * P, j0=jtot, ntiles=ntiles,
                                  rng=rg, c=c, g=g))
            jtot += ntiles
    plan.jtot = jtot
    plan.tot16 = jtot * 8

    nslots = jtot * P
    gidx = np.zeros((NCORES, P, plan.tot16), np.int16)
    deg_col = np.full((NCORES, nslots), DEG_PAD, np.float32)
    dest_arr = np.full((NCORES, nslots), -1, np.int64)  # rel to range base
    cnts = np.zeros((NCORES, max(1, len(plan.segs))), np.int32)
    for m in range(NCORES):
        r_l, c_g, b = cores[m]["r_l"], cores[m]["c_g"], cores[m]["bounds"]
        idx16 = np.full(nslots, -1, np.int16)
        for si, seg in enumerate(plan.segs):
            g = seg["g"]
            lo, hi = int(b[g]), int(b[g + 1])
            n = hi - lo
            if n == 0:
                # still need >= 1 valid index (dummy row 0, zero S row)
                idx16[seg["j0"] * P] = 0
                cnts[m, si] = 1
                continue
            cnts[m, si] = n
            s0 = seg["j0"] * P
            cg = c_g[lo:hi]
            idx16[s0:s0 + n] = (cg - seg["base"]).astype(np.int16)
            deg_col[m, s0:s0 + n] = deg[cg]
            dest_arr[m, s0:s0 + n] = (r_l[lo:hi]
                                      - seg["rng"] * RNG_DTS * P)
        w = idx16.reshape(plan.tot16, 16).T
        gidx[m] = np.tile(w, (8, 1))

    # common per-tile S frames (dmin/nd = union over cores, within the
    # RNG_DTS*128-wide range -> nd <= 512 always)
    da = dest_arr.reshape(NCORES, jtot, P)
    da_min = np.where(da < 0, 10 ** 9, da).min(axis=(0, 2))
    da_max = da.max(axis=(0, 2))
    plan.dmin = da_min.astype(np.int64)
    plan.nd = (da_max - da_min + 1).astype(np.int64)
    assert (plan.nd >= 1).all() and (plan.nd <= RNG_DTS * P).all()
    plan.s0 = np.zeros(jtot + 1, np.int64)
    np.cumsum(plan.nd, out=plan.s0[1:])
    plan.stot = int(plan.s0[-1])

    # S matrices
    s_pack = np.zeros((NCORES, P, plan.stot), ml_dtypes.bfloat16)
    ar = np.arange(P)
    for m in range(NCORES):
        for t in range(jtot):
            dl = da[m, t]
            v = dl >= 0
            if not v.any():
                continue
            blk = np.zeros((P, int(plan.nd[t])), np.float32)
            blk[ar[v], dl[v] - plan.dmin[t]] = 1.0
            s_pack[m, :, plan.s0[t]:plan.s0[t + 1]] = blk

    deg_nat = np.full((NCORES, P, rt), 1.0, np.float32)
    for m in range(NCORES):
        d = np.full(rt * P, 1.0, np.float32)
        d[:ns] = deg[m * ns:(m + 1) * ns]
        deg_nat[m] = d.reshape(rt, P).T

    deg_col = deg_col.reshape(NCORES, jtot, P).transpose(0, 2, 1)

    # per-range tile spans (S streamed per range)
    plan.rng_tiles = []
    seg_by_rng = {}
    for si, seg in enumerate(plan.segs):
        seg_by_rng.setdefault(seg["rng"], []).append(si)
    t = 0
    for rg in range(nrng):
        t0 = t
        for si in seg_by_rng.get(rg, []):
            t += plan.segs[si]["ntiles"]
        plan.rng_tiles.append((t0, t))
    plan.seg_by_rng = seg_by_rng
    plan.swmax = max((int(plan.s0[t1] - plan.s0[t0])
                      for t0, t1 in plan.rng_tiles if t1 > t0), default=1)
    plan.nmax = max(s["n"] for s in plan.segs)

    data = dict(gidx=gidx, deg_col=np.ascontiguousarray(deg_col),
                s_pack=s_pack, deg_nat=deg_nat, cnts=cnts)
    return plan, data


# ----------------------------------------------------------------------------
# Device program (identical for all cores)
# ----------------------------------------------------------------------------

def _build_nc(n_nodes: int, plan: Plan):
    ns, rt, nch, nrng = plan.ns, plan.rt, plan.nch, plan.nrng
    nc = bacc.Bacc("TRN2", target_bir_lowering=False, debug=False,
                   num_devices=NCORES, num_swdge_queues=NQ)

    x_d = nc.dram_tensor("x", [n_nodes, P], F32, kind="ExternalInput").ap()
    wt_d = nc.dram_tensor("wt", [P, P], F32, kind="ExternalInput").ap()
    gix_d = nc.dram_tensor("gidx", [P, plan.tot16], I16,
                           kind="ExternalInput").ap()
    dcol_d = nc.dram_tensor("deg_col", [P, plan.jtot], F32,
                            kind="ExternalInput").ap()
    dnat_d = nc.dram_tensor("deg_nat", [P, rt], F32,
                            kind="ExternalInput").ap()
    s_d = nc.dram_tensor("s_pack", [P, plan.stot], BF16,
                         kind="ExternalInput").ap()
    cnt_d = nc.dram_tensor("cnts", [1, max(1, len(plan.segs))],
                           mybir.dt.int32, kind="ExternalInput").ap()
    out_d = nc.dram_tensor("out", [rt * P, P], F32, kind="ExternalOutput").ap()

    pw = RNG_DTS * P
    with tile.TileContext(nc) as tc:
        nc.gpsimd.load_library(library_config.mlp)
        with (
            tc.tile_pool(name="const", bufs=1) as cpool,
            tc.tile_pool(name="gbuf", bufs=4) as gpool,
            tc.tile_pool(name="gb16", bufs=4) as bpool,
            tc.tile_pool(name="sbuf_s", bufs=2) as spool,
            tc.tile_pool(name="fin", bufs=4) as fpool,
            tc.tile_pool(name="pacc", bufs=4, space="PSUM") as papool,
            tc.tile_pool(name="pout", bufs=2, space="PSUM") as popool,
        ):
            # W.T cast to bf16 on device (SWDGE dtype-cast DMA)
            wt_sb = cpool.tile([P, P], BF16)
            nc.gpsimd.dma_start(out=wt_sb[:], in_=wt_d[:, :])
            gidx_sb = cpool.tile([P, plan.tot16], I16)
            nc.sync.dma_start(out=gidx_sb[:], in_=gix_d[:, :])

            dcol_sb = cpool.tile([P, plan.jtot], F32)
            nc.sync.dma_start(out=dcol_sb[:], in_=dcol_d[:, :])
            nc.scalar.activation(dcol_sb[:], dcol_sb[:],
                                 mybir.ActivationFunctionType.Sqrt)
            d_col = cpool.tile([P, plan.jtot], F32)
            nc.vector.reciprocal(d_col[:], dcol_sb[:])

            dnat_sb = cpool.tile([P, rt], F32)
            nc.sync.dma_start(out=dnat_sb[:], in_=dnat_d[:, :])
            nc.scalar.activation(dnat_sb[:], dnat_sb[:],
                                 mybir.ActivationFunctionType.Sqrt)
            d_nat = cpool.tile([P, rt], F32)
            nc.vector.reciprocal(d_nat[:], dnat_sb[:])

            zcol = cpool.tile([1, P], BF16)
            nc.vector.memset(zcol[:], 0.0)
            zrow = cpool.tile([1, pw], BF16)
            nc.vector.memset(zrow[:], 0.0)

            cnt_sb = cpool.tile([1, max(1, len(plan.segs))], mybir.dt.int32)
            nc.sync.dma_start(out=cnt_sb[:], in_=cnt_d[:, :])
            cnt_regs = [nc.gpsimd.alloc_register(f"cntr{i}")
                        for i in range(NQ)]

            # one-time memset of the gather buffers: pad slots never touched
            # by any gather keep finite values (their S rows are zero, so
            # they contribute exactly 0; this only guards against NaN bit
            # patterns in virgin SBUF reaching a matmul).
            for _ in range(4):
                gz = gpool.tile([P, plan.nmax], F32, tag="g")
                nc.vector.memset(gz[:], 0.0)

            for rg in range(nrng):
                t0, t1 = plan.rng_tiles[rg]
                if t1 == t0:
                    continue
                sw0, sw1 = int(plan.s0[t0]), int(plan.s0[t1])
                s_sb = spool.tile([P, plan.swmax], BF16, tag="s_sb")
                nc.sync.dma_start(out=s_sb[:, :sw1 - sw0],
                                  in_=s_d[:, sw0:sw1])

                pt = papool.tile([P, pw], F32, tag="pacc")
                nc.tensor.matmul(pt[:], lhsT=zcol[:], rhs=zrow[:],
                                 start=True, stop=False,
                                 skip_group_check=True)

                segs_rng = plan.seg_by_rng.get(rg, [])
                for k, si in enumerate(segs_rng):
                    seg = plan.segs[si]
                    jseg, nseg = seg["ntiles"], seg["n"]
                    g = gpool.tile([P, plan.nmax], F32, tag="g")
                    g3 = g[:, :nseg].rearrange("p (j f) -> p j f", f=P)
                    span = min(CH_SPAN, n_nodes - seg["base"])
                    creg = cnt_regs[si % NQ]
                    nc.gpsimd.reg_load(creg, cnt_sb[0:1, si:si + 1])
                    nc.gpsimd.dma_gather(
                        g3, x_d[seg["base"]:seg["base"] + span, :],
                        gidx_sb[:, seg["t16_0"]:seg["t16_0"] + jseg * 8],
                        nseg, creg, P, single_packet=False,
                        queue_num=si % NQ,
                    )
                    gb = bpool.tile([P, plan.nmax], BF16, tag="gb")
                    gb3 = gb[:, :nseg].rearrange("p (j f) -> p j f", f=P)
                    dsl = d_col[:, seg["j0"]:seg["j0"] + jseg]
                    nc.vector.tensor_mul(
                        gb3, g3, dsl[:, :, None].to_broadcast([P, jseg, P]))
                    for jj in range(jseg):
                        t = seg["j0"] + jj
                        dmin, nd = int(plan.dmin[t]), int(plan.nd[t])
                        sa = int(plan.s0[t]) - sw0
                        is_last = (k == len(segs_rng) - 1 and jj == jseg - 1)
                        nc.tensor.matmul(
                            pt[:, dmin:dmin + nd],
                            lhsT=gb[:, jj * P:(jj + 1) * P],
                            rhs=s_sb[:, sa:sa + nd],
                            start=False, stop=is_last,
                            skip_group_check=True,
                        )

                for dl in range(min(RNG_DTS, rt - rg * RNG_DTS)):
                    dt = rg * RNG_DTS + dl
                    aggt = fpool.tile([P, P], BF16, tag="aggt")
                    nc.vector.tensor_copy(aggt[:], pt[:, dl * P:(dl + 1) * P])
                    op = popool.tile([P, P], F32, tag="op")
                    nc.tensor.matmul(op[:], lhsT=aggt[:], rhs=wt_sb[:],
                                     start=True, stop=True)
                    ob = fpool.tile([P, P], F32, tag="ob")
                    nc.vector.tensor_scalar_mul(ob[:], op[:],
                                                d_nat[:, dt:dt + 1])
                    nc.sync.dma_start(out=out_d[dt * P:(dt + 1) * P, :],
                                      in_=ob[:])
    nc.compile()
    return nc


# ----------------------------------------------------------------------------
# Entry point
# ----------------------------------------------------------------------------

_CACHE: dict = {}


def _prepare(X, W, edge_index):
    X = np.ascontiguousarray(np.asarray(X, dtype=np.float32))
    W = np.asarray(W, dtype=np.float32)
    edge_index = np.asarray(edge_index)
    n = X.shape[0]
    plan, data = _preprocess(edge_index, n)
    key = (n, plan.jtot, plan.stot, tuple(s["n"] for s in plan.segs))
    if key not in _CACHE:
        _CACHE.clear()
        _CACHE[key] = _build_nc(n, plan)
    nc = _CACHE[key]
    wt = np.ascontiguousarray(W.T)
    in_maps = [
        {
            "x": X,
            "wt": wt,
            "gidx": np.ascontiguousarray(data["gidx"][m]),
            "deg_col": np.ascontiguousarray(data["deg_col"][m]),
            "deg_nat": np.ascontiguousarray(data["deg_nat"][m]),
            "s_pack": np.ascontiguousarray(data["s_pack"][m]),
            "cnts": np.ascontiguousarray(data["cnts"][m][None, :]),
        }
        for m in range(NCORES)
    ]
    return nc, in_maps, plan


def kernel(X, W, edge_index):
    nc, in_maps, plan = _prepare(X, W, edge_index)
    res = run_bass_kernel_spmd(nc, in_maps, core_ids=list(range(NCORES)))
    ns = plan.ns
    return np.concatenate([res.results[m]["out"][:ns] for m in range(NCORES)],
                          axis=0)


# revision 15
# speedup vs baseline: 4.0320x; 2.0162x over previous
"""GCNConv on 8 Trainium2 NeuronCores (Bass/Tile, SPMD).

out = D^-1/2 (A+I) D^-1/2 (X @ W.T),   deg = in-degree(col) + 1

Math refactoring (exact in real arithmetic):
    agg[r]  = sum_{e: dst=r} d[col_e] * X[col_e]      (self loop = edge (r,r))
    out[r]  = d[r] * (agg[r] @ W.T)                   (d = deg^-1/2)

Distribution: destinations (rows) are sharded across the 8 cores (12500
each); each core processes the edges whose destination lands in its shard.
X and W are replicated so any core can read any source row.

Device algorithm per core (one SPMD program; per-core index tables are
padded into a common, max-over-cores structure so SPMD is preserved):

  * Edge slots: edges (+ self loops) are grouped into segments by (range of
    RNG_DTS dest-tiles, source-chunk c of 25000 rows), sorted by destination
    inside each segment and packed densely (slot i of a gather lives at SBUF
    partition i%128, free block i//128).  Trailing pad entries use index -1
    and a per-core valid-count register, so padding costs zero descriptors.
  * Gather: one `dma_gather` (int16 indices relative to the chunk base) per
    segment pulls the 512-byte X rows of its slots.  The per-descriptor cost
    is a per-SWDGE-queue drain wall (~9 ns/desc on one queue); the gathers
    round-robin over all 4 SWDGE queues, which overlaps their drains and
    brings the aggregate rate to ~2.3 ns/desc.
  * Scale: one DVE multiply per segment applies d[col] per slot
    (per-partition scalar broadcast over the 128 features), writing a bf16
    copy of the tile; d is computed on device from the integer degree counts
    (ACT sqrt + DVE reciprocal).  Pad slots are killed by zero S rows (and a
    one-time buffer memset guarantees no NaN garbage on first use).
  * Segmented sum via PE: per 128-slot tile, a host-built 0/1 selection
    matrix S (slots x dests, bf16) routes slots to destinations:
    psum[feat, dest] += g_tile.T @ S_tile (both operands bf16, 1 cycle/row),
    accumulating into a range-wide one-bank PSUM tile [128, RNG_DTS*128].
  * Finalize per dest-tile: copy PSUM->SBUF (bf16), matmul with W.T
    (bf16, contraction over features), scale by d[dest] (per-partition
    scalar, f32), DMA out.

The host does index marshaling only (bucketing, sorting, degree counts, 0/1
selection structure); all floating-point math on X/W runs on device.
"""

import math
import os

import numpy as np
import ml_dtypes

_ABL = os.environ.get("KERNEL_ABL", "full")

import concourse.bacc as bacc
import concourse.mybir as mybir
import concourse.tile as tile
from concourse.bass_utils import run_bass_kernel_spmd
from concourse import library_config

NCORES = 8
P = 128
CH_SPAN = 25000          # source rows per gather chunk (int16-indexable)
RNG_DTS = 4              # dest-tiles per range (psum tile = 1 bank = 512 f32)
NQ = 4                   # SWDGE queues (gather drains overlap across queues)
SUB_MAX = 24             # max tiles per dma_gather (ring + SBUF friendly)
GBUFS = 10               # gather buffers in flight (Pool runs ahead of drains)
DMA_SCRATCH = 32768      # SWDGE descriptor-ring carveout bytes per partition
DEG_PAD = 1.0e30         # pad degree -> d ~ 0

F32 = mybir.dt.float32
BF16 = mybir.dt.bfloat16
I16 = mybir.dt.int16


class Plan:
    pass


# ----------------------------------------------------------------------------
# Host-side index marshaling
# ----------------------------------------------------------------------------

def _preprocess(edge_index: np.ndarray, n_nodes: int):
    ns = n_nodes // NCORES
    rt = math.ceil(ns / P)
    nch = math.ceil(n_nodes / CH_SPAN)
    nrng = math.ceil(rt / RNG_DTS)

    row = np.asarray(edge_index[0]).astype(np.int64)
    col = np.asarray(edge_index[1]).astype(np.int64)
    deg = (np.bincount(col, minlength=n_nodes) + 1).astype(np.float32)

    core = row // ns
    cores = []
    for m in range(NCORES):
        sel = core == m
        r_l = row[sel] - m * ns
        c_g = col[sel]
        r_l = np.concatenate([r_l, np.arange(ns, dtype=np.int64)])
        c_g = np.concatenate([c_g, np.arange(ns, dtype=np.int64) + m * ns])
        rg = r_l // (RNG_DTS * P)
        ch = np.minimum(c_g // CH_SPAN, nch - 1)
        order = np.lexsort((c_g, r_l, ch, rg))
        r_l, c_g = r_l[order], c_g[order]
        code = rg[order] * nch + ch[order]
        bounds = np.searchsorted(code, np.arange(nrng * nch + 1))
        cores.append(dict(r_l=r_l, c_g=c_g, bounds=bounds))

    # segment tile counts: max over cores (packed, no per-dt padding)
    plan = Plan()
    plan.ns, plan.rt, plan.nch, plan.nrng = ns, rt, nch, nrng
    plan.segs = []
    jtot = 0
    for rg in range(nrng):
        for c in range(nch):
            g = rg * nch + c
            ntiles = 0
            for m in range(NCORES):
                b = cores[m]["bounds"]
                ntiles = max(ntiles, (int(b[g + 1] - b[g]) + P - 1) // P)
            if ntiles == 0:
                continue
            plan.segs.append(dict(base=c * CH_SPAN, t16_0=jtot * 8,
                                  n=ntiles * P, j0=jtot, ntiles=ntiles,
                                  rng=rg, c=c, g=g))
            jtot += ntiles
    plan.jtot = jtot
    plan.tot16 = jtot * 8

    # sub-gather entries: split oversized segments into <= SUB_MAX-tile
    # gathers (keeps per-queue rings shallow and gather buffers small)
    plan.entries = []
    for si, seg in enumerate(plan.segs):
        nsub = (seg["ntiles"] + SUB_MAX - 1) // SUB_MAX
        starts = [seg["ntiles"] * s // nsub for s in range(nsub + 1)]
        for s in range(nsub):
            j0e = seg["j0"] + starts[s]
            nt = starts[s + 1] - starts[s]
            plan.entries.append(dict(
                seg=si, base=seg["base"], rng=seg["rng"], j0=j0e,
                ntiles=nt, n=nt * P, t16_0=j0e * 8, slot0=j0e * P))

    nslots = jtot * P
    gidx = np.zeros((NCORES, P, plan.tot16), np.int16)
    deg_col = np.full((NCORES, nslots), DEG_PAD, np.float32)
    dest_arr = np.full((NCORES, nslots), -1, np.int64)  # rel to range base
    seg_cnt = np.zeros((NCORES, len(plan.segs)), np.int64)
    idx16s = []
    for m in range(NCORES):
        r_l, c_g, b = cores[m]["r_l"], cores[m]["c_g"], cores[m]["bounds"]
        idx16 = np.full(nslots, -1, np.int16)
        for si, seg in enumerate(plan.segs):
            g = seg["g"]
            lo, hi = int(b[g]), int(b[g + 1])
            n = hi - lo
            seg_cnt[m, si] = n
            if n == 0:
                continue
            s0 = seg["j0"] * P
            cg = c_g[lo:hi]
            idx16[s0:s0 + n] = (cg - seg["base"]).astype(np.int16)
            deg_col[m, s0:s0 + n] = deg[cg]
            dest_arr[m, s0:s0 + n] = (r_l[lo:hi]
                                      - seg["rng"] * RNG_DTS * P)
        idx16s.append(idx16)

    cnts = np.zeros((NCORES, max(1, len(plan.entries))), np.int32)
    for m in range(NCORES):
        idx16 = idx16s[m]
        for ei, ent in enumerate(plan.entries):
            seg = plan.segs[ent["seg"]]
            off = ent["slot0"] - seg["j0"] * P
            n = int(np.clip(seg_cnt[m, ent["seg"]] - off, 0, ent["n"]))
            if n == 0:
                # still need >= 1 valid index (dummy row 0, zero S row)
                idx16[ent["slot0"]] = 0
                n = 1
            cnts[m, ei] = n
        w = idx16.reshape(plan.tot16, 16).T
        gidx[m] = np.tile(w, (8, 1))

    # common per-tile S frames (dmin/nd = union over cores, within the
    # RNG_DTS*128-wide range -> nd <= 512 always)
    da = dest_arr.reshape(NCORES, jtot, P)
    da_min = np.where(da < 0, 10 ** 9, da).min(axis=(0, 2))
    da_max = da.max(axis=(0, 2))
    plan.dmin = da_min.astype(np.int64)
    plan.nd = (da_max - da_min + 1).astype(np.int64)
    assert (plan.nd >= 1).all() and (plan.nd <= RNG_DTS * P).all()
    plan.s0 = np.zeros(jtot + 1, np.int64)
    np.cumsum(plan.nd, out=plan.s0[1:])
    plan.stot = int(plan.s0[-1])

    # S matrices
    s_pack = np.zeros((NCORES, P, plan.stot), ml_dtypes.bfloat16)
    ar = np.arange(P)
    for m in range(NCORES):
        for t in range(jtot):
            dl = da[m, t]
            v = dl >= 0
            if not v.any():
                continue
            blk = np.zeros((P, int(plan.nd[t])), np.float32)
            blk[ar[v], dl[v] - plan.dmin[t]] = 1.0
            s_pack[m, :, plan.s0[t]:plan.s0[t + 1]] = blk

    deg_nat = np.full((NCORES, P, rt), 1.0, np.float32)
    for m in range(NCORES):
        d = np.full(rt * P, 1.0, np.float32)
        d[:ns] = deg[m * ns:(m + 1) * ns]
        deg_nat[m] = d.reshape(rt, P).T

    deg_col = deg_col.reshape(NCORES, jtot, P).transpose(0, 2, 1)

    # per-range tile spans (S, gidx, deg_col streamed per range)
    plan.rng_tiles = []
    ent_by_rng = {}
    for ei, ent in enumerate(plan.entries):
        ent_by_rng.setdefault(ent["rng"], []).append(ei)
    seg_by_rng = {}
    for si, seg in enumerate(plan.segs):
        seg_by_rng.setdefault(seg["rng"], []).append(si)
    t = 0
    for rg in range(nrng):
        t0 = t
        for si in seg_by_rng.get(rg, []):
            t += plan.segs[si]["ntiles"]
        plan.rng_tiles.append((t0, t))
    plan.ent_by_rng = ent_by_rng
    plan.swmax = max((int(plan.s0[t1] - plan.s0[t0])
                      for t0, t1 in plan.rng_tiles if t1 > t0), default=1)
    plan.rtmax = max((t1 - t0 for t0, t1 in plan.rng_tiles), default=1)
    plan.nmax = max(e["n"] for e in plan.entries)

    data = dict(gidx=gidx, deg_col=np.ascontiguousarray(deg_col),
                s_pack=s_pack, deg_nat=deg_nat, cnts=cnts)
    return plan, data


# ----------------------------------------------------------------------------
# Device program (identical for all cores)
# ----------------------------------------------------------------------------

def _build_nc(n_nodes: int, plan: Plan):
    ns, rt, nch, nrng = plan.ns, plan.rt, plan.nch, plan.nrng
    nc = bacc.Bacc("TRN2", target_bir_lowering=False, debug=False,
                   num_devices=NCORES, num_swdge_queues=NQ,
                   dynamic_dma_scratch_size=DMA_SCRATCH)

    x_d = nc.dram_tensor("x", [n_nodes, P], F32, kind="ExternalInput").ap()
    wt_d = nc.dram_tensor("wt", [P, P], F32, kind="ExternalInput").ap()
    gix_d = nc.dram_tensor("gidx", [P, plan.tot16], I16,
                           kind="ExternalInput").ap()
    dcol_d = nc.dram_tensor("deg_col", [P, plan.jtot], F32,
                            kind="ExternalInput").ap()
    dnat_d = nc.dram_tensor("deg_nat", [P, rt], F32,
                            kind="ExternalInput").ap()
    s_d = nc.dram_tensor("s_pack", [P, plan.stot], BF16,
                         kind="ExternalInput").ap()
    cnt_d = nc.dram_tensor("cnts", [1, max(1, len(plan.segs))],
                           mybir.dt.int32, kind="ExternalInput").ap()
    out_d = nc.dram_tensor("out", [rt * P, P], F32, kind="ExternalOutput").ap()

    pw = RNG_DTS * P
    with tile.TileContext(nc) as tc:
        nc.gpsimd.load_library(library_config.mlp)
        with (
            tc.tile_pool(name="const", bufs=1) as cpool,
            tc.tile_pool(name="gbuf", bufs=GBUFS) as gpool,
            tc.tile_pool(name="gb16", bufs=4) as bpool,
            tc.tile_pool(name="sbuf_s", bufs=2) as spool,
            tc.tile_pool(name="gix", bufs=2) as xpool,
            tc.tile_pool(name="dcl", bufs=2) as dpool,
            tc.tile_pool(name="fin", bufs=4) as fpool,
            tc.tile_pool(name="pacc", bufs=4, space="PSUM") as papool,
            tc.tile_pool(name="pout", bufs=2, space="PSUM") as popool,
        ):
            # W.T cast to bf16 on device (SWDGE dtype-cast DMA)
            wt_sb = cpool.tile([P, P], BF16)
            nc.gpsimd.dma_start(out=wt_sb[:], in_=wt_d[:, :])

            dnat_sb = cpool.tile([P, rt], F32)
            nc.sync.dma_start(out=dnat_sb[:], in_=dnat_d[:, :])
            nc.scalar.activation(dnat_sb[:], dnat_sb[:],
                                 mybir.ActivationFunctionType.Sqrt)
            d_nat = cpool.tile([P, rt], F32)
            nc.vector.reciprocal(d_nat[:], dnat_sb[:])

            zcol = cpool.tile([1, P], BF16)
            nc.vector.memset(zcol[:], 0.0)
            zrow = cpool.tile([1, pw], BF16)
            nc.vector.memset(zrow[:], 0.0)

            cnt_sb = cpool.tile([1, max(1, len(plan.entries))], mybir.dt.int32)
            nc.sync.dma_start(out=cnt_sb[:], in_=cnt_d[:, :])
            cnt_regs = [nc.gpsimd.alloc_register(f"cntr{i}")
                        for i in range(NQ)]

            # one-time memset of the gather buffers: pad slots never touched
            # by any gather keep finite values (their S rows are zero, so
            # they contribute exactly 0; this only guards against NaN bit
            # patterns in virgin SBUF reaching a matmul).
            nbuf = SUB_MAX * P
            g_fixed = None
            for _ in range(GBUFS):
                gz = gpool.tile([P, nbuf], F32, tag="g")
                nc.vector.memset(gz[:], 0.0)
                g_fixed = gz

            gi = 0
            for rg in range(nrng):
                t0, t1 = plan.rng_tiles[rg]
                if t1 == t0:
                    continue
                rtiles = t1 - t0
                sw0, sw1 = int(plan.s0[t0]), int(plan.s0[t1])
                do_route = _ABL not in ("gather", "nomm")
                s_sb = spool.tile([P, plan.swmax], BF16, tag="s_sb")
                if do_route:
                    nc.sync.dma_start(out=s_sb[:, :sw1 - sw0],
                                      in_=s_d[:, sw0:sw1])
                gixr = xpool.tile([P, plan.rtmax * 8], I16, tag="gix")
                nc.sync.dma_start(out=gixr[:, :rtiles * 8],
                                  in_=gix_d[:, t0 * 8:t1 * 8])
                dv = dpool.tile([P, plan.rtmax], F32, tag="dv")
                nc.sync.dma_start(out=dv[:, :rtiles],
                                  in_=dcol_d[:, t0:t1])
                nc.scalar.activation(dv[:, :rtiles], dv[:, :rtiles],
                                     mybir.ActivationFunctionType.Sqrt)
                nc.vector.reciprocal(dv[:, :rtiles], dv[:, :rtiles])

                pt = papool.tile([P, pw], F32, tag="pacc")
                nc.tensor.matmul(pt[:], lhsT=zcol[:], rhs=zrow[:],
                                 start=True, stop=not do_route,
                                 skip_group_check=True)

                ents_rng = plan.ent_by_rng.get(rg, [])
                for k, ei in enumerate(ents_rng):
                    ent = plan.entries[ei]
                    jseg, nseg = ent["ntiles"], ent["n"]
                    g = (g_fixed if _ABL == "nogather"
                         else gpool.tile([P, nbuf], F32, tag="g"))
                    g3 = g[:, :nseg].rearrange("p (j f) -> p j f", f=P)
                    span = min(CH_SPAN, n_nodes - ent["base"])
                    jo = ent["j0"] - t0
                    if _ABL != "nogather":
                        creg = cnt_regs[gi % NQ]
                        nc.gpsimd.reg_load(creg, cnt_sb[0:1, ei:ei + 1])
                        nc.gpsimd.dma_gather(
                            g3, x_d[ent["base"]:ent["base"] + span, :],
                            gixr[:, jo * 8:(jo + jseg) * 8],
                            nseg, creg, P, single_packet=False,
                            queue_num=gi % NQ,
                        )
                    gi += 1
                    if _ABL == "gather":
                        continue
                    gb = bpool.tile([P, nbuf], BF16, tag="gb")
                    gb3 = gb[:, :nseg].rearrange("p (j f) -> p j f", f=P)
                    dsl = dv[:, jo:jo + jseg]
                    if _ABL != "noscale":
                        nc.vector.tensor_mul(
                            gb3, g3,
                            dsl[:, :, None].to_broadcast([P, jseg, P]))
                    if _ABL == "nomm":
                        continue
                    for jj in range(jseg):
                        t = ent["j0"] + jj
                        dmin, nd = int(plan.dmin[t]), int(plan.nd[t])
                        sa = int(plan.s0[t]) - sw0
                        is_last = (k == len(ents_rng) - 1 and jj == jseg - 1)
                        nc.tensor.matmul(
                            pt[:, dmin:dmin + nd],
                            lhsT=gb[:, jj * P:(jj + 1) * P],
                            rhs=s_sb[:, sa:sa + nd],
                            start=False, stop=is_last,
                            skip_group_check=True,
                        )

                if _ABL == "gather":
                    continue
                for dl in range(min(RNG_DTS, rt - rg * RNG_DTS)):
                    dt = rg * RNG_DTS + dl
                    aggt = fpool.tile([P, P], BF16, tag="aggt")
                    nc.vector.tensor_copy(aggt[:], pt[:, dl * P:(dl + 1) * P])
                    op = popool.tile([P, P], F32, tag="op")
                    nc.tensor.matmul(op[:], lhsT=aggt[:], rhs=wt_sb[:],
                                     start=True, stop=True)
                    ob = fpool.tile([P, P], F32, tag="ob")
                    nc.vector.tensor_scalar_mul(ob[:], op[:],
                                                d_nat[:, dt:dt + 1])
                    nc.sync.dma_start(out=out_d[dt * P:(dt + 1) * P, :],
                                      in_=ob[:])
    nc.compile()
    return nc


# ----------------------------------------------------------------------------
# Entry point
# ----------------------------------------------------------------------------

_CACHE: dict = {}


def _prepare(X, W, edge_index):
    X = np.ascontiguousarray(np.asarray(X, dtype=np.float32))
    W = np.asarray(W, dtype=np.float32)
    edge_index = np.asarray(edge_index)
    n = X.shape[0]
    plan, data = _preprocess(edge_index, n)
    key = (n, plan.jtot, plan.stot, tuple(s["n"] for s in plan.segs))
    if key not in _CACHE:
        _CACHE.clear()
        _CACHE[key] = _build_nc(n, plan)
    nc = _CACHE[key]
    wt = np.ascontiguousarray(W.T)
    in_maps = [
        {
            "x": X,
            "wt": wt,
            "gidx": np.ascontiguousarray(data["gidx"][m]),
            "deg_col": np.ascontiguousarray(data["deg_col"][m]),
            "deg_nat": np.ascontiguousarray(data["deg_nat"][m]),
            "s_pack": np.ascontiguousarray(data["s_pack"][m]),
            "cnts": np.ascontiguousarray(data["cnts"][m][None, :]),
        }
        for m in range(NCORES)
    ]
    return nc, in_maps, plan


def kernel(X, W, edge_index):
    nc, in_maps, plan = _prepare(X, W, edge_index)
    res = run_bass_kernel_spmd(nc, in_maps, core_ids=list(range(NCORES)))
    ns = plan.ns
    return np.concatenate([res.results[m]["out"][:ns] for m in range(NCORES)],
                          axis=0)
